# revision 18
# baseline (speedup 1.0000x reference)
"""Mixtral MoE (top-2 of 8 experts, GLU) on 8 Trainium2 cores.  v4.

Structure (per core, SPMD-uniform):
  - tokens laid out as a flat [128, 8, cap] block; "chunks" (<=512 tokens)
    for stage 1 and 128-token "blocks" for stage 2, each statically mapped
    to a weight slot (wslot).  The standard layout is a 2048-token main run
    (wslot 0) + one small spill chunk (wslot 1), so each expert's weights
    stream once per f-tile instead of once per 512-slot (4x less SBUF-write
    DMA traffic -> fewer PE stalls from port contention).
  - loop: ft outer; stage 1 (all chunks) -> hmid; stage 2 (all blocks)
    accumulates into a bf16 oacc; final ft adds in fp32, scales by coef and
    streams out.
"""

import numpy as np
import ml_dtypes

B, S, H, F, E, TOPK = 4, 2048, 1024, 3584, 8, 2
T = B * S
NCORES = 8
NFT = 7                # F tiles
FT = F // NFT          # 512
FC = FT // 128         # 4
NH = H // 512          # 2
BF16 = ml_dtypes.bfloat16

_compiled = {}


def _ceil_div(a, b):
    return -(-a // b)


# --------------------------------------------------------------------------
# device kernel
# --------------------------------------------------------------------------

def _build_nc(layout):
    """layout: tuple of (chunk_size, wslot) pairs; chunk starts must keep
    128-token blocks within a single wslot (sizes multiple of 128 except
    possibly the last chunk of a wslot run)."""
    import concourse.tile as tile
    import concourse.mybir as mybir
    from concourse import bacc

    sizes = [c[0] for c in layout]
    wslots = [c[1] for c in layout]
    nw = max(wslots) + 1
    cap = sum(sizes)
    offs = np.concatenate([[0], np.cumsum(sizes)]).astype(int)
    # stage-2 blocks: global 128-grid; each block must lie inside one chunk
    blocks = []   # (tok0, mw, wslot)
    for (sz, w), o in zip(layout, offs[:-1]):
        t0 = 0
        while t0 < sz:
            mw = min(128, sz - t0)
            blocks.append((int(o + t0), mw, w))
            t0 += mw
    nblk = len(blocks)

    nc = bacc.Bacc("TRN2", target_bir_lowering=False, debug=False,
                   num_devices=NCORES)
    xt = nc.dram_tensor("xt", [128, 8, cap], mybir.dt.bfloat16,
                        kind="ExternalInput")
    # fc-major so per-fc startup chunks are contiguous in DRAM
    w1t = nc.dram_tensor("w1t", [nw, NFT, 128, FC, 8, 128],
                         mybir.dt.bfloat16, kind="ExternalInput")
    v1t = nc.dram_tensor("v1t", [nw, NFT, 128, FC, 8, 128],
                         mybir.dt.bfloat16, kind="ExternalInput")
    w2 = nc.dram_tensor("w2", [nw, NFT, 128, 4, H], mybir.dt.bfloat16,
                        kind="ExternalInput")
    coef = nc.dram_tensor("coef", [128, nblk], mybir.dt.float32,
                          kind="ExternalInput")
    yout = nc.dram_tensor("yout", [128, nblk, H], mybir.dt.bfloat16,
                          kind="ExternalOutput")

    with tile.TileContext(nc) as tc:
        with (
            tc.tile_pool(name="xpool", bufs=1) as xpool,
            tc.tile_pool(name="wpool", bufs=2) as wpool,
            tc.tile_pool(name="hpool", bufs=2) as hpool,
            tc.tile_pool(name="spool", bufs=2) as spool,
            tc.tile_pool(name="opool", bufs=1) as opool,
            tc.tile_pool(name="tpool", bufs=2) as tpool,
            tc.tile_pool(name="cpool", bufs=1) as cpool,
            tc.tile_pool(name="ps1", bufs=2, space="PSUM") as ps1,
            tc.tile_pool(name="ps2", bufs=2, space="PSUM") as ps2,
            tc.tile_pool(name="pso", bufs=4, space="PSUM") as psop,
        ):
            # PE warm-up burst: dummy matmuls during the initial DMA fill so
            # HAM un-throttles before real work.
            wu = cpool.tile([128, 128], mybir.dt.bfloat16)
            nc.gpsimd.memset(wu[:], 0.0)
            wups = ps1.tile([128, 512], mybir.dt.float32, tag="p1")
            for _ in range(60):
                nc.tensor.matmul(wups[:, :128], wu[:], wu[:],
                                 start=True, stop=True)

            coefs = cpool.tile([128, nblk], mybir.dt.float32)
            xts = xpool.tile([128, 8, cap], mybir.dt.bfloat16, tag="xts")
            # bf16 running accumulator over f-tiles (fp32 finish in tpool)
            oacc = opool.tile([128, nblk, H], mybir.dt.bfloat16, tag="oacc")

            for ft in range(NFT):
                w1s, v1s, w2s = [], [], []
                for w in range(nw):
                    wb = 2 if w == 0 else 1
                    w1s.append(wpool.tile([128, FC, 8, 128],
                                          mybir.dt.bfloat16,
                                          tag=f"w1s{w}", name=f"w1s{w}",
                                          bufs=wb))
                    v1s.append(wpool.tile([128, FC, 8, 128],
                                          mybir.dt.bfloat16,
                                          tag=f"v1s{w}", name=f"v1s{w}",
                                          bufs=wb))
                    w2s.append(wpool.tile([128, 4, H], mybir.dt.bfloat16,
                                          tag=f"w2s{w}", name=f"w2s{w}",
                                          bufs=wb))
                if ft == 0:
                    # startup: first token chunk + first weight fc-chunk
                    # land first; later chunks stream in while the first
                    # segment computes (stage-2 interleaving keeps the
                    # early bandwidth demand low)
                    nc.sync.dma_start(xts[:, :, 0:int(offs[1])],
                                      xt[:, :, 0:int(offs[1])])
                    nc.sync.dma_start(w1s[0][:, 0], w1t[0, ft, :, 0])
                    nc.sync.dma_start(v1s[0][:, 0], v1t[0, ft, :, 0])
                    for fc in range(1, FC):
                        nc.sync.dma_start(w1s[0][:, fc], w1t[0, ft, :, fc])
                        nc.sync.dma_start(v1s[0][:, fc], v1t[0, ft, :, fc])
                    nc.sync.dma_start(w2s[0][:], w2[0, ft])
                    for ((sz, _), o) in list(zip(layout, offs[:-1]))[1:]:
                        nc.sync.dma_start(xts[:, :, o:o + sz],
                                          xt[:, :, o:o + sz])
                    for w in range(1, nw):
                        nc.sync.dma_start(w1s[w][:], w1t[w, ft])
                        nc.sync.dma_start(v1s[w][:], v1t[w, ft])
                        nc.sync.dma_start(w2s[w][:], w2[w, ft])
                    nc.sync.dma_start(coefs[:], coef[:])
                else:
                    for w in range(nw):
                        nc.sync.dma_start(w1s[w][:], w1t[w, ft])
                        nc.sync.dma_start(v1s[w][:], v1t[w, ft])
                        nc.sync.dma_start(w2s[w][:], w2[w, ft])

                hmid = hpool.tile([128, FC, cap], mybir.dt.bfloat16,
                                  tag="hmid")
                # per segment: stage 1 over fc, then its stage-2 blocks --
                # interleaving stage 2 keeps the PE busy on already-loaded
                # data while later token chunks / weights stream in
                for (sz, w), o in zip(layout, offs[:-1]):
                    t0, tl = int(o), int(sz)
                    for fc in range(FC):
                        p1 = ps1.tile([128, 512], mybir.dt.float32)
                        p2 = ps2.tile([128, 512], mybir.dt.float32)
                        for hs in range(8):
                            nc.tensor.matmul(
                                p1[:, :tl], w1s[w][:, fc, hs],
                                xts[:, hs, t0:t0 + tl],
                                start=(hs == 0), stop=(hs == 7))
                        for hs in range(8):
                            nc.tensor.matmul(
                                p2[:, :tl], v1s[w][:, fc, hs],
                                xts[:, hs, t0:t0 + tl],
                                start=(hs == 0), stop=(hs == 7))
                        sil = spool.tile([128, 512], mybir.dt.float32)
                        nc.scalar.activation(
                            sil[:, :tl], p1[:, :tl],
                            mybir.ActivationFunctionType.Silu)
                        nc.vector.tensor_mul(
                            hmid[:, fc, t0:t0 + tl], sil[:, :tl],
                            p2[:, :tl])

                    nb = _ceil_div(tl, 128)
                    b0 = t0 // 128
                    for m in range(nb):
                        bi = b0 + m
                        mw = min(128, tl - m * 128)
                        msl = slice(t0 + m * 128, t0 + m * 128 + mw)
                        pos = [psop.tile([128, 512], mybir.dt.float32,
                                         tag="po", name=f"po{n}")
                               for n in range(NH)]
                        for fc in range(FC):  # lhsT reused across n chunks
                            for n in range(NH):
                                nc.tensor.matmul(
                                    pos[n][:mw], hmid[:, fc, msl],
                                    w2s[w][:, fc, n * 512:(n + 1) * 512],
                                    start=(fc == 0), stop=(fc == FC - 1))
                        if ft < NFT - 1:
                            for n in range(NH):
                                osl = oacc[:mw, bi, n * 512:(n + 1) * 512]
                                if ft == 0:
                                    nc.scalar.copy(osl, pos[n][:mw])
                                else:
                                    nc.vector.tensor_add(osl, osl,
                                                         pos[n][:mw])
                        else:
                            # finish: add + scale, single per-block bf16 DMA
                            # so the tail pipeline keeps up with the MMs
                            fin = tpool.tile([128, H], mybir.dt.bfloat16,
                                             bufs=4)
                            for n in range(NH):
                                nsl = slice(n * 512, (n + 1) * 512)
                                nc.vector.tensor_add(
                                    fin[:mw, nsl], oacc[:mw, bi, nsl],
                                    pos[n][:mw])
                            nc.vector.tensor_scalar_mul(
                                fin[:mw, :], fin[:mw, :],
                                coefs[:mw, bi:bi + 1])
                            nc.sync.dma_start(yout[:mw, bi, :], fin[:mw])

    nc.compile()
    return nc


def _get_nc(layout):
    if layout not in _compiled:
        _compiled[layout] = _build_nc(layout)
    return _compiled[layout]


# --------------------------------------------------------------------------
# host side: routing, packing, layout
# --------------------------------------------------------------------------

def _route(x, router_w):
    """Top-2 router, matching the reference (jax on CPU if available)."""
    try:
        import jax
        import jax.numpy as jnp
        cpu = jax.devices("cpu")[0]
        with jax.default_device(cpu):
            xl = jax.device_put(jnp.asarray(x), cpu)
            rw = jax.device_put(jnp.asarray(router_w), cpu)
            logits = xl @ rw.T
            scores = jax.nn.softmax(logits.astype(jnp.float32), axis=-1)
            ew, ei = jax.lax.top_k(scores, TOPK)
            ew = ew / ew.sum(axis=-1, keepdims=True)
            return np.asarray(ew, np.float32), np.asarray(ei, np.int64)
    except Exception:
        logits = x.astype(np.float32) @ router_w.astype(np.float32).T
        m = logits.max(axis=-1, keepdims=True)
        p = np.exp(logits - m)
        scores = (p / p.sum(axis=-1, keepdims=True)).astype(np.float32)
        i1 = scores.argmax(axis=-1)
        s2 = scores.copy()
        s2[np.arange(T), i1] = -np.inf
        i2 = s2.argmax(axis=-1)
        wa = scores[np.arange(T), i1]
        wb = scores[np.arange(T), i2]
        tot = wa + wb
        ew = np.stack([wa / tot, wb / tot], axis=-1).astype(np.float32)
        ei = np.stack([i1, i2], axis=-1).astype(np.int64)
        return ew, ei


def _pack(counts):
    """Big+spill packing.

    Layout (SPMD-uniform): 4x(512, wslot 0) main run + one (spill, wslot 1)
    chunk.  Each expert (desc count) gets its own core: first min(c_e, 2048)
    tokens fill the main run; overflow is cut into <=spill pieces placed in
    other cores' spill chunk.  Cores without a spill piece duplicate their
    main expert in wslot 1 with zero coef.

    Returns (layout, per_core_runs) with per_core_runs[c] a list of
    (wslot, tok_off, expert, n_tokens), or (None, None) if infeasible."""
    order = [int(e) for e in np.argsort(-counts) if counts[e] > 0]
    if len(order) > NCORES:
        return None, None
    spills = []
    runs = [[] for _ in range(NCORES)]
    for c, e in enumerate(order):
        rem = int(counts[e])
        runs[c].append((0, 0, e, min(rem, 2048)))
        if rem > 2048:
            spills.append([e, rem - 2048])
    if not spills:
        return tuple([(512, 0)] * 4), runs
    for spill_sz in (64, 128, 256, 512):
        pieces = []
        for e, rem in spills:
            n = _ceil_div(rem, spill_sz)
            pieces += [(e, min(spill_sz, rem - i * spill_sz))
                       for i in range(n)]
        if len(pieces) <= NCORES:
            for c, (e, n) in enumerate(pieces):
                runs[c].append((1, 2048, e, n))
            layout = tuple([(512, 0)] * 4 + [(spill_sz, 1)])
            return layout, runs
    return None, None


def _pack_fallback(counts):
    """General fallback: greedy bin-pack of experts onto 8 copies of a
    static slot template; each slot gets its own weight slot (old
    behaviour, weights re-streamed per slot)."""
    for tpl in ((512, 512, 512, 512, 512),
                (512,) * 6, (512,) * 8, (1024,) * 4, (2048,) * 3):
        slots = []
        for c in range(NCORES):
            for i, sz in enumerate(tpl):
                slots.append([sz, c, i, None, 0])
        free = sorted(range(len(slots)), key=lambda i: -slots[i][0])
        ok = True
        for e in np.argsort(-counts):
            rem = int(counts[e])
            while rem > 0:
                fit = [i for i in free if slots[i][0] >= rem]
                if fit:
                    pick = min(fit, key=lambda i: slots[i][0])
                elif free:
                    pick = free[0]
                else:
                    ok = False
                    break
                free.remove(pick)
                take = min(rem, slots[pick][0])
                slots[pick][3] = int(e)
                slots[pick][4] = take
                rem -= take
            if not ok:
                break
        if not ok:
            continue
        offs = np.concatenate([[0], np.cumsum(tpl)]).astype(int)
        runs = [[] for _ in range(NCORES)]
        for sz, c, i, e, used in slots:
            if e is not None:
                runs[c].append((i, int(offs[i]), e, used))
        layout = tuple((sz, i) for i, sz in enumerate(tpl))
        return layout, runs
    raise AssertionError("no feasible packing")


def _to_bf16(a):
    """Fast float32 -> bfloat16 with round-to-nearest-even."""
    u = np.ascontiguousarray(a, np.float32).view(np.uint32)
    r = ((u + np.uint32(0x7FFF) + ((u >> np.uint32(16)) & np.uint32(1)))
         >> np.uint32(16)).astype(np.uint16)
    return r.view(BF16)


def _prep_weights(w1, v1, w2):
    """Per-expert device layouts (bf16).

    w1t/v1t: [E][NFT,128,FC,8,128]  elem [ft,p,fc,hs,fl] =
                 W[ft*FT+fc*128+fl, hs*128+p]   (fc-major, contiguous chunks)
    w2     : [E][NFT,128,4,H]   elem [ft,p,fc,h] = w2[ft*FT+fc*128+p, h]
    """
    w1t, v1t, w2d = [], [], []
    for e in range(E):
        for src, dst in ((w1, w1t), (v1, v1t)):
            a = _to_bf16(src[e])                      # [F, H]
            a = np.ascontiguousarray(a.T)             # [H, F]
            a = a.reshape(8, 128, NFT, FC, 128).transpose(2, 1, 3, 0, 4)
            dst.append(np.ascontiguousarray(a))
        b = _to_bf16(w2[e])                           # [F, H]
        b = b.reshape(NFT, 4, 128, H).transpose(0, 2, 1, 3)
        w2d.append(np.ascontiguousarray(b))
    return w1t, v1t, w2d


def _forward(hidden_states, router_w, w1, v1, w2, trace=False):
    from concourse.bass_utils import run_bass_kernel_spmd

    x = np.ascontiguousarray(np.asarray(hidden_states, np.float32)).reshape(T, H)
    router_w = np.asarray(router_w, np.float32)
    w1 = np.asarray(w1, np.float32)
    v1 = np.asarray(v1, np.float32)
    w2 = np.asarray(w2, np.float32)

    ew, ei = _route(x, router_w)
    counts = np.bincount(ei.ravel(), minlength=E)
    layout, per_core_runs = _pack(counts)
    if layout is None:
        layout, per_core_runs = _pack_fallback(counts)
    sizes = [c[0] for c in layout]
    cap = sum(sizes)
    nw = max(w for _, w in layout) + 1
    offs = np.concatenate([[0], np.cumsum(sizes)]).astype(int)
    # block grid (must match device)
    blocks = []
    for (sz, w), o in zip(layout, offs[:-1]):
        t0 = 0
        while t0 < sz:
            mw = min(128, sz - t0)
            blocks.append((int(o + t0), mw, w))
            t0 += mw
    nblk = len(blocks)

    # per-expert assignment lists (token ids + weights), then cursors
    flat_e = ei.ravel()
    flat_w = ew.ravel().astype(np.float32)
    order = np.argsort(flat_e, kind="stable")
    toks_s = (order // TOPK).astype(np.int64)
    ws_s = flat_w[order]
    starts = np.concatenate([[0], np.cumsum(counts)]).astype(int)
    cursor = starts[:-1].copy()

    w1t_pre, v1t_pre, w2_pre = _prep_weights(w1, v1, w2)
    xbf = _to_bf16(x)  # [T, H] bf16

    in_maps = []
    core_lists = []  # per core: list of (tok_off, ids) for scatter
    for c in range(NCORES):
        xt_np = np.zeros((128, 8, cap), BF16)
        w1t_np = np.zeros((nw, NFT, 128, FC, 8, 128), BF16)
        v1t_np = np.zeros((nw, NFT, 128, FC, 8, 128), BF16)
        w2_np = np.zeros((nw, NFT, 128, 4, H), BF16)
        coef_np = np.zeros((128, nblk), np.float32)
        lists = []
        filled = set()
        for wslot, tok_off, e, used in per_core_runs[c]:
            ids = toks_s[cursor[e]:cursor[e] + used]
            ws = ws_s[cursor[e]:cursor[e] + used]
            cursor[e] += used
            L = used
            xg = np.ascontiguousarray(xbf[ids].T)     # [H, L]
            xt_np[:, :, tok_off:tok_off + L] = \
                xg.reshape(8, 128, L).transpose(1, 0, 2)
            nm = _ceil_div(L, 128)
            wpad = np.zeros(nm * 128, np.float32)
            wpad[:L] = ws
            b0 = tok_off // 128  # runs start 128-aligned
            coef_np[:, b0:b0 + nm] = wpad.reshape(nm, 128).T
            w1t_np[wslot] = w1t_pre[e]
            v1t_np[wslot] = v1t_pre[e]
            w2_np[wslot] = w2_pre[e]
            filled.add(wslot)
            lists.append((tok_off, ids))
        # unused weight slots: duplicate expert 0 weights (coef stays 0,
        # so the computed garbage is multiplied by zero -- but weights must
        # be finite)
        for wslot in range(nw):
            if wslot not in filled:
                w1t_np[wslot] = w1t_pre[0]
                v1t_np[wslot] = v1t_pre[0]
                w2_np[wslot] = w2_pre[0]
        core_lists.append(lists)
        in_maps.append({"xt": xt_np, "w1t": w1t_np, "v1t": v1t_np,
                        "w2": w2_np, "coef": coef_np})
    assert (cursor == starts[1:]).all()

    nc = _get_nc(layout)
    if trace:
        _install_profile_shim()
    res = run_bass_kernel_spmd(nc, in_maps, list(range(NCORES)), trace=trace)

    out = np.zeros((T, H), np.float32)
    for c in range(NCORES):
        y = res.results[c]["yout"]  # [128, nblk, H] bf16
        yflat = np.asarray(y, np.float32).transpose(1, 0, 2) \
            .reshape(nblk * 128, H)
        for tok_off, ids in core_lists[c]:
            L = len(ids)
            out[ids] += yflat[tok_off:tok_off + L]
    return out.reshape(B, S, H), res


def kernel(hidden_states, router_w, w1, v1, w2):
    out, _ = _forward(hidden_states, router_w, w1, v1, w2, trace=False)
    return out


def _install_profile_shim():
    """The agent image's antenv lacks axon_hooks; register the NTFF
    profile hook from trn_agent_boot so trace=True works."""
    import sys
    import types
    if "antenv.axon_hooks" in sys.modules:
        return
    holder = {}
    mod = types.ModuleType("antenv.axon_hooks")
    mod.set_axon_ntff_profile_hook = lambda h: holder.__setitem__("h", h)
    mod.get_axon_ntff_profile_hook = lambda: holder.get("h")
    sys.modules["antenv.axon_hooks"] = mod
    try:
        from trn_agent_boot.trn_boot import _ntff_profile_via_ctypes
        hook = _ntff_profile_via_ctypes("/opt/axon/libaxon_pjrt.so")
        mod.set_axon_ntff_profile_hook(hook)
    except Exception as exc:  # pragma: no cover
        print(f"profile shim failed: {exc}")


# revision 22
# speedup vs baseline: 1.1819x; 1.1819x over previous
"""Mixtral MoE (top-2 of 8 experts, GLU) on 8 Trainium2 cores.  v4.

Structure (per core, SPMD-uniform):
  - tokens laid out as a flat [128, 8, cap] block; "chunks" (<=512 tokens)
    for stage 1 and 128-token "blocks" for stage 2, each statically mapped
    to a weight slot (wslot).  The standard layout is a 2048-token main run
    (wslot 0) + one small spill chunk (wslot 1), so each expert's weights
    stream once per f-tile instead of once per 512-slot (4x less SBUF-write
    DMA traffic -> fewer PE stalls from port contention).
  - loop: ft outer; stage 1 (all chunks) -> hmid; stage 2 (all blocks)
    accumulates into a bf16 oacc; final ft adds in fp32, scales by coef and
    streams out.
"""

import numpy as np
import ml_dtypes

B, S, H, F, E, TOPK = 4, 2048, 1024, 3584, 8, 2
T = B * S
NCORES = 8
NFT = 7                # F tiles
FT = F // NFT          # 512
FC = FT // 128         # 4
NH = H // 512          # 2
BF16 = ml_dtypes.bfloat16

_compiled = {}


def _ceil_div(a, b):
    return -(-a // b)


# --------------------------------------------------------------------------
# device kernel
# --------------------------------------------------------------------------

def _build_nc(layout):
    """layout: tuple of (chunk_size, wslot) pairs; chunk starts must keep
    128-token blocks within a single wslot (sizes multiple of 128 except
    possibly the last chunk of a wslot run)."""
    import concourse.tile as tile
    import concourse.mybir as mybir
    from concourse import bacc

    sizes = [c[0] for c in layout]
    wslots = [c[1] for c in layout]
    nw = max(wslots) + 1
    cap = sum(sizes)
    offs = np.concatenate([[0], np.cumsum(sizes)]).astype(int)
    # stage-2 blocks: global 128-grid; each block must lie inside one chunk
    blocks = []   # (tok0, mw, wslot)
    for (sz, w), o in zip(layout, offs[:-1]):
        t0 = 0
        while t0 < sz:
            mw = min(128, sz - t0)
            blocks.append((int(o + t0), mw, w))
            t0 += mw
    nblk = len(blocks)

    nc = bacc.Bacc("TRN2", target_bir_lowering=False, debug=False,
                   num_devices=NCORES)
    # one contiguous DRAM tensor per token chunk (fast whole-block DMA)
    xtc = [nc.dram_tensor(f"xt{i}", [128, 8, sz], mybir.dt.bfloat16,
                          kind="ExternalInput")
           for i, sz in enumerate(sizes)]
    # fc-major so per-fc startup chunks are contiguous in DRAM
    w1t = nc.dram_tensor("w1t", [nw, NFT, 128, FC, 8, 128],
                         mybir.dt.bfloat16, kind="ExternalInput")
    v1t = nc.dram_tensor("v1t", [nw, NFT, 128, FC, 8, 128],
                         mybir.dt.bfloat16, kind="ExternalInput")
    w2 = nc.dram_tensor("w2", [nw, NFT, 128, 4, H], mybir.dt.bfloat16,
                        kind="ExternalInput")
    coef = nc.dram_tensor("coef", [128, nblk], mybir.dt.float32,
                          kind="ExternalInput")
    yout = nc.dram_tensor("yout", [128, nblk, H], mybir.dt.bfloat16,
                          kind="ExternalOutput")

    with tile.TileContext(nc) as tc:
        with (
            tc.tile_pool(name="xpool", bufs=1) as xpool,
            tc.tile_pool(name="wpool", bufs=2) as wpool,
            tc.tile_pool(name="hpool", bufs=2) as hpool,
            tc.tile_pool(name="spool", bufs=2) as spool,
            tc.tile_pool(name="opool", bufs=1) as opool,
            tc.tile_pool(name="tpool", bufs=2) as tpool,
            tc.tile_pool(name="cpool", bufs=1) as cpool,
            tc.tile_pool(name="ps1", bufs=2, space="PSUM") as ps1,
            tc.tile_pool(name="ps2", bufs=2, space="PSUM") as ps2,
            tc.tile_pool(name="pso", bufs=4, space="PSUM") as psop,
        ):
            # PE warm-up burst: dummy matmuls during the initial DMA fill so
            # HAM un-throttles before real work.
            wu = cpool.tile([128, 128], mybir.dt.bfloat16)
            nc.gpsimd.memset(wu[:], 0.0)
            wups = ps1.tile([128, 512], mybir.dt.float32, tag="p1")
            for _ in range(60):
                nc.tensor.matmul(wups[:, :128], wu[:], wu[:],
                                 start=True, stop=True)

            coefs = cpool.tile([128, nblk], mybir.dt.float32)
            xts = xpool.tile([128, 8, cap], mybir.dt.bfloat16, tag="xts")
            # bf16 running accumulator over f-tiles (fp32 finish in tpool)
            oacc = opool.tile([128, nblk, H], mybir.dt.bfloat16, tag="oacc")

            for ft in range(NFT):
                w1s, v1s, w2s = [], [], []
                for w in range(nw):
                    wb = 2 if w == 0 else 1
                    w1s.append(wpool.tile([128, FC, 8, 128],
                                          mybir.dt.bfloat16,
                                          tag=f"w1s{w}", name=f"w1s{w}",
                                          bufs=wb))
                    v1s.append(wpool.tile([128, FC, 8, 128],
                                          mybir.dt.bfloat16,
                                          tag=f"v1s{w}", name=f"v1s{w}",
                                          bufs=wb))
                    w2s.append(wpool.tile([128, 4, H], mybir.dt.bfloat16,
                                          tag=f"w2s{w}", name=f"w2s{w}",
                                          bufs=wb))
                if ft == 0:
                    # startup: first token chunk + first weight fc-chunk
                    # land first; later chunks stream in while the first
                    # segment computes (stage-2 interleaving keeps the
                    # early bandwidth demand low)
                    nc.sync.dma_start(xts[:, :, 0:int(offs[1])], xtc[0][:])
                    nc.sync.dma_start(w1s[0][:, 0], w1t[0, ft, :, 0])
                    nc.sync.dma_start(v1s[0][:, 0], v1t[0, ft, :, 0])
                    for fc in range(1, FC):
                        nc.sync.dma_start(w1s[0][:, fc], w1t[0, ft, :, fc])
                        nc.sync.dma_start(v1s[0][:, fc], v1t[0, ft, :, fc])
                    nc.sync.dma_start(w2s[0][:], w2[0, ft])
                    for i, ((sz, _), o) in list(
                            enumerate(zip(layout, offs[:-1])))[1:]:
                        nc.sync.dma_start(xts[:, :, o:o + sz], xtc[i][:])
                    for w in range(1, nw):
                        nc.sync.dma_start(w1s[w][:], w1t[w, ft])
                        nc.sync.dma_start(v1s[w][:], v1t[w, ft])
                        nc.sync.dma_start(w2s[w][:], w2[w, ft])
                    nc.sync.dma_start(coefs[:], coef[:])
                else:
                    for w in range(nw):
                        nc.sync.dma_start(w1s[w][:], w1t[w, ft])
                        nc.sync.dma_start(v1s[w][:], v1t[w, ft])
                        nc.sync.dma_start(w2s[w][:], w2[w, ft])

                hmid = hpool.tile([128, FC, cap], mybir.dt.bfloat16,
                                  tag="hmid")
                # per segment: stage 1 over fc, then its stage-2 blocks --
                # interleaving stage 2 keeps the PE busy on already-loaded
                # data while later token chunks / weights stream in
                for (sz, w), o in zip(layout, offs[:-1]):
                    t0, tl = int(o), int(sz)
                    for fc in range(FC):
                        p1 = ps1.tile([128, 512], mybir.dt.float32)
                        p2 = ps2.tile([128, 512], mybir.dt.float32)
                        for hs in range(8):
                            nc.tensor.matmul(
                                p1[:, :tl], w1s[w][:, fc, hs],
                                xts[:, hs, t0:t0 + tl],
                                start=(hs == 0), stop=(hs == 7))
                        for hs in range(8):
                            nc.tensor.matmul(
                                p2[:, :tl], v1s[w][:, fc, hs],
                                xts[:, hs, t0:t0 + tl],
                                start=(hs == 0), stop=(hs == 7))
                        sil = spool.tile([128, 512], mybir.dt.float32)
                        nc.scalar.activation(
                            sil[:, :tl], p1[:, :tl],
                            mybir.ActivationFunctionType.Silu)
                        nc.vector.tensor_mul(
                            hmid[:, fc, t0:t0 + tl], sil[:, :tl],
                            p2[:, :tl])

                    nb = _ceil_div(tl, 128)
                    b0 = t0 // 128
                    for m in range(nb):
                        bi = b0 + m
                        mw = min(128, tl - m * 128)
                        msl = slice(t0 + m * 128, t0 + m * 128 + mw)
                        pos = [psop.tile([128, 512], mybir.dt.float32,
                                         tag="po", name=f"po{n}")
                               for n in range(NH)]
                        for fc in range(FC):  # lhsT reused across n chunks
                            for n in range(NH):
                                nc.tensor.matmul(
                                    pos[n][:mw], hmid[:, fc, msl],
                                    w2s[w][:, fc, n * 512:(n + 1) * 512],
                                    start=(fc == 0), stop=(fc == FC - 1))
                        if ft < NFT - 1:
                            for n in range(NH):
                                osl = oacc[:mw, bi, n * 512:(n + 1) * 512]
                                if ft == 0:
                                    nc.scalar.copy(osl, pos[n][:mw])
                                else:
                                    nc.vector.tensor_add(osl, osl,
                                                         pos[n][:mw])
                        else:
                            # finish: add + scale, single per-block bf16 DMA
                            # so the tail pipeline keeps up with the MMs
                            fin = tpool.tile([128, H], mybir.dt.bfloat16,
                                             bufs=4)
                            for n in range(NH):
                                nsl = slice(n * 512, (n + 1) * 512)
                                nc.vector.tensor_add(
                                    fin[:mw, nsl], oacc[:mw, bi, nsl],
                                    pos[n][:mw])
                            nc.vector.tensor_scalar_mul(
                                fin[:mw, :], fin[:mw, :],
                                coefs[:mw, bi:bi + 1])
                            nc.sync.dma_start(yout[:mw, bi, :], fin[:mw])

    nc.compile()
    return nc


def _get_nc(layout):
    if layout not in _compiled:
        _compiled[layout] = _build_nc(layout)
    return _compiled[layout]


# --------------------------------------------------------------------------
# host side: routing, packing, layout
# --------------------------------------------------------------------------

def _route(x, router_w):
    """Top-2 router, matching the reference (jax on CPU if available)."""
    try:
        import jax
        import jax.numpy as jnp
        cpu = jax.devices("cpu")[0]
        with jax.default_device(cpu):
            xl = jax.device_put(jnp.asarray(x), cpu)
            rw = jax.device_put(jnp.asarray(router_w), cpu)
            logits = xl @ rw.T
            scores = jax.nn.softmax(logits.astype(jnp.float32), axis=-1)
            ew, ei = jax.lax.top_k(scores, TOPK)
            ew = ew / ew.sum(axis=-1, keepdims=True)
            return np.asarray(ew, np.float32), np.asarray(ei, np.int64)
    except Exception:
        logits = x.astype(np.float32) @ router_w.astype(np.float32).T
        m = logits.max(axis=-1, keepdims=True)
        p = np.exp(logits - m)
        scores = (p / p.sum(axis=-1, keepdims=True)).astype(np.float32)
        i1 = scores.argmax(axis=-1)
        s2 = scores.copy()
        s2[np.arange(T), i1] = -np.inf
        i2 = s2.argmax(axis=-1)
        wa = scores[np.arange(T), i1]
        wb = scores[np.arange(T), i2]
        tot = wa + wb
        ew = np.stack([wa / tot, wb / tot], axis=-1).astype(np.float32)
        ei = np.stack([i1, i2], axis=-1).astype(np.int64)
        return ew, ei


def _pack(counts):
    """Big+spill packing.

    Layout (SPMD-uniform): 4x(512, wslot 0) main run + one (spill, wslot 1)
    chunk.  Each expert (desc count) gets its own core: first min(c_e, 2048)
    tokens fill the main run; overflow is cut into <=spill pieces placed in
    other cores' spill chunk.  Cores without a spill piece duplicate their
    main expert in wslot 1 with zero coef.

    Returns (layout, per_core_runs) with per_core_runs[c] a list of
    (wslot, tok_off, expert, n_tokens), or (None, None) if infeasible."""
    order = [int(e) for e in np.argsort(-counts) if counts[e] > 0]
    if len(order) > NCORES:
        return None, None
    spills = []
    runs = [[] for _ in range(NCORES)]
    for c, e in enumerate(order):
        rem = int(counts[e])
        runs[c].append((0, 0, e, min(rem, 2048)))
        if rem > 2048:
            spills.append([e, rem - 2048])
    main_chunks = [(128, 0), (384, 0), (512, 0), (512, 0), (512, 0)]
    if not spills:
        return tuple(main_chunks), runs
    for spill_sz in (64, 128, 256, 512):
        pieces = []
        for e, rem in spills:
            n = _ceil_div(rem, spill_sz)
            pieces += [(e, min(spill_sz, rem - i * spill_sz))
                       for i in range(n)]
        if len(pieces) <= NCORES:
            for c, (e, n) in enumerate(pieces):
                runs[c].append((1, 2048, e, n))
            layout = tuple(main_chunks + [(spill_sz, 1)])
            return layout, runs
    return None, None


def _pack_fallback(counts):
    """General fallback: greedy bin-pack of experts onto 8 copies of a
    static slot template; each slot gets its own weight slot (old
    behaviour, weights re-streamed per slot)."""
    for tpl in ((512, 512, 512, 512, 512),
                (512,) * 6, (512,) * 8, (1024,) * 4, (2048,) * 3):
        slots = []
        for c in range(NCORES):
            for i, sz in enumerate(tpl):
                slots.append([sz, c, i, None, 0])
        free = sorted(range(len(slots)), key=lambda i: -slots[i][0])
        ok = True
        for e in np.argsort(-counts):
            rem = int(counts[e])
            while rem > 0:
                fit = [i for i in free if slots[i][0] >= rem]
                if fit:
                    pick = min(fit, key=lambda i: slots[i][0])
                elif free:
                    pick = free[0]
                else:
                    ok = False
                    break
                free.remove(pick)
                take = min(rem, slots[pick][0])
                slots[pick][3] = int(e)
                slots[pick][4] = take
                rem -= take
            if not ok:
                break
        if not ok:
            continue
        offs = np.concatenate([[0], np.cumsum(tpl)]).astype(int)
        runs = [[] for _ in range(NCORES)]
        for sz, c, i, e, used in slots:
            if e is not None:
                runs[c].append((i, int(offs[i]), e, used))
        layout = tuple((sz, i) for i, sz in enumerate(tpl))
        return layout, runs
    raise AssertionError("no feasible packing")


def _to_bf16(a):
    """Fast float32 -> bfloat16 with round-to-nearest-even."""
    u = np.ascontiguousarray(a, np.float32).view(np.uint32)
    r = ((u + np.uint32(0x7FFF) + ((u >> np.uint32(16)) & np.uint32(1)))
         >> np.uint32(16)).astype(np.uint16)
    return r.view(BF16)


def _prep_weights(w1, v1, w2):
    """Per-expert device layouts (bf16).

    w1t/v1t: [E][NFT,128,FC,8,128]  elem [ft,p,fc,hs,fl] =
                 W[ft*FT+fc*128+fl, hs*128+p]   (fc-major, contiguous chunks)
    w2     : [E][NFT,128,4,H]   elem [ft,p,fc,h] = w2[ft*FT+fc*128+p, h]
    """
    w1t, v1t, w2d = [], [], []
    for e in range(E):
        for src, dst in ((w1, w1t), (v1, v1t)):
            a = _to_bf16(src[e])                      # [F, H]
            a = np.ascontiguousarray(a.T)             # [H, F]
            a = a.reshape(8, 128, NFT, FC, 128).transpose(2, 1, 3, 0, 4)
            dst.append(np.ascontiguousarray(a))
        b = _to_bf16(w2[e])                           # [F, H]
        b = b.reshape(NFT, 4, 128, H).transpose(0, 2, 1, 3)
        w2d.append(np.ascontiguousarray(b))
    return w1t, v1t, w2d


def _forward(hidden_states, router_w, w1, v1, w2, trace=False):
    from concourse.bass_utils import run_bass_kernel_spmd

    x = np.ascontiguousarray(np.asarray(hidden_states, np.float32)).reshape(T, H)
    router_w = np.asarray(router_w, np.float32)
    w1 = np.asarray(w1, np.float32)
    v1 = np.asarray(v1, np.float32)
    w2 = np.asarray(w2, np.float32)

    ew, ei = _route(x, router_w)
    counts = np.bincount(ei.ravel(), minlength=E)
    layout, per_core_runs = _pack(counts)
    if layout is None:
        layout, per_core_runs = _pack_fallback(counts)
    sizes = [c[0] for c in layout]
    cap = sum(sizes)
    nw = max(w for _, w in layout) + 1
    offs = np.concatenate([[0], np.cumsum(sizes)]).astype(int)
    # block grid (must match device)
    blocks = []
    for (sz, w), o in zip(layout, offs[:-1]):
        t0 = 0
        while t0 < sz:
            mw = min(128, sz - t0)
            blocks.append((int(o + t0), mw, w))
            t0 += mw
    nblk = len(blocks)

    # per-expert assignment lists (token ids + weights), then cursors
    flat_e = ei.ravel()
    flat_w = ew.ravel().astype(np.float32)
    order = np.argsort(flat_e, kind="stable")
    toks_s = (order // TOPK).astype(np.int64)
    ws_s = flat_w[order]
    starts = np.concatenate([[0], np.cumsum(counts)]).astype(int)
    cursor = starts[:-1].copy()

    w1t_pre, v1t_pre, w2_pre = _prep_weights(w1, v1, w2)
    xbf = _to_bf16(x)  # [T, H] bf16

    in_maps = []
    core_lists = []  # per core: list of (tok_off, ids) for scatter
    for c in range(NCORES):
        xt_np = np.zeros((128, 8, cap), BF16)
        w1t_np = np.zeros((nw, NFT, 128, FC, 8, 128), BF16)
        v1t_np = np.zeros((nw, NFT, 128, FC, 8, 128), BF16)
        w2_np = np.zeros((nw, NFT, 128, 4, H), BF16)
        coef_np = np.zeros((128, nblk), np.float32)
        lists = []
        filled = set()
        for wslot, tok_off, e, used in per_core_runs[c]:
            ids = toks_s[cursor[e]:cursor[e] + used]
            ws = ws_s[cursor[e]:cursor[e] + used]
            cursor[e] += used
            L = used
            xg = np.ascontiguousarray(xbf[ids].T)     # [H, L]
            xt_np[:, :, tok_off:tok_off + L] = \
                xg.reshape(8, 128, L).transpose(1, 0, 2)
            nm = _ceil_div(L, 128)
            wpad = np.zeros(nm * 128, np.float32)
            wpad[:L] = ws
            b0 = tok_off // 128  # runs start 128-aligned
            coef_np[:, b0:b0 + nm] = wpad.reshape(nm, 128).T
            w1t_np[wslot] = w1t_pre[e]
            v1t_np[wslot] = v1t_pre[e]
            w2_np[wslot] = w2_pre[e]
            filled.add(wslot)
            lists.append((tok_off, ids))
        # unused weight slots: duplicate expert 0 weights (coef stays 0,
        # so the computed garbage is multiplied by zero -- but weights must
        # be finite)
        for wslot in range(nw):
            if wslot not in filled:
                w1t_np[wslot] = w1t_pre[0]
                v1t_np[wslot] = v1t_pre[0]
                w2_np[wslot] = w2_pre[0]
        core_lists.append(lists)
        im = {"w1t": w1t_np, "v1t": v1t_np, "w2": w2_np, "coef": coef_np}
        for i, ((sz, _), o) in enumerate(zip(layout, offs[:-1])):
            im[f"xt{i}"] = np.ascontiguousarray(xt_np[:, :, o:o + sz])
        in_maps.append(im)
    assert (cursor == starts[1:]).all()

    nc = _get_nc(layout)
    if trace:
        _install_profile_shim()
    res = run_bass_kernel_spmd(nc, in_maps, list(range(NCORES)), trace=trace)

    out = np.zeros((T, H), np.float32)
    for c in range(NCORES):
        y = res.results[c]["yout"]  # [128, nblk, H] bf16
        yflat = np.asarray(y, np.float32).transpose(1, 0, 2) \
            .reshape(nblk * 128, H)
        for tok_off, ids in core_lists[c]:
            L = len(ids)
            out[ids] += yflat[tok_off:tok_off + L]
    return out.reshape(B, S, H), res


def kernel(hidden_states, router_w, w1, v1, w2):
    out, _ = _forward(hidden_states, router_w, w1, v1, w2, trace=False)
    return out


def _install_profile_shim():
    """The agent image's antenv lacks axon_hooks; register the NTFF
    profile hook from trn_agent_boot so trace=True works."""
    import sys
    import types
    if "antenv.axon_hooks" in sys.modules:
        return
    holder = {}
    mod = types.ModuleType("antenv.axon_hooks")
    mod.set_axon_ntff_profile_hook = lambda h: holder.__setitem__("h", h)
    mod.get_axon_ntff_profile_hook = lambda: holder.get("h")
    sys.modules["antenv.axon_hooks"] = mod
    try:
        from trn_agent_boot.trn_boot import _ntff_profile_via_ctypes
        hook = _ntff_profile_via_ctypes("/opt/axon/libaxon_pjrt.so")
        mod.set_axon_ntff_profile_hook(hook)
    except Exception as exc:  # pragma: no cover
        print(f"profile shim failed: {exc}")


# revision 23
# speedup vs baseline: 1.1972x; 1.0129x over previous
"""Mixtral MoE (top-2 of 8 experts, GLU) on 8 Trainium2 cores.  v4.

Structure (per core, SPMD-uniform):
  - tokens laid out as a flat [128, 8, cap] block; "chunks" (<=512 tokens)
    for stage 1 and 128-token "blocks" for stage 2, each statically mapped
    to a weight slot (wslot).  The standard layout is a 2048-token main run
    (wslot 0) + one small spill chunk (wslot 1), so each expert's weights
    stream once per f-tile instead of once per 512-slot (4x less SBUF-write
    DMA traffic -> fewer PE stalls from port contention).
  - loop: ft outer; stage 1 (all chunks) -> hmid; stage 2 (all blocks)
    accumulates into a bf16 oacc; final ft adds in fp32, scales by coef and
    streams out.
"""

import numpy as np
import ml_dtypes

B, S, H, F, E, TOPK = 4, 2048, 1024, 3584, 8, 2
T = B * S
NCORES = 8
NFT = 7                # F tiles
FT = F // NFT          # 512
FC = FT // 128         # 4
NH = H // 512          # 2
BF16 = ml_dtypes.bfloat16

_compiled = {}


def _ceil_div(a, b):
    return -(-a // b)


# --------------------------------------------------------------------------
# device kernel
# --------------------------------------------------------------------------

def _build_nc(layout):
    """layout: tuple of (chunk_size, wslot) pairs; chunk starts must keep
    128-token blocks within a single wslot (sizes multiple of 128 except
    possibly the last chunk of a wslot run)."""
    import concourse.tile as tile
    import concourse.mybir as mybir
    from concourse import bacc

    sizes = [c[0] for c in layout]
    wslots = [c[1] for c in layout]
    nw = max(wslots) + 1
    cap = sum(sizes)
    offs = np.concatenate([[0], np.cumsum(sizes)]).astype(int)
    # stage-2 blocks: global 128-grid; each block must lie inside one chunk
    blocks = []   # (tok0, mw, wslot)
    for (sz, w), o in zip(layout, offs[:-1]):
        t0 = 0
        while t0 < sz:
            mw = min(128, sz - t0)
            blocks.append((int(o + t0), mw, w))
            t0 += mw
    nblk = len(blocks)

    nc = bacc.Bacc("TRN2", target_bir_lowering=False, debug=False,
                   num_devices=NCORES)
    xt = nc.dram_tensor("xt", [128, 8, cap], mybir.dt.bfloat16,
                        kind="ExternalInput")
    # fc-major so per-fc startup chunks are contiguous in DRAM
    w1t = nc.dram_tensor("w1t", [nw, NFT, 128, FC, 8, 128],
                         mybir.dt.bfloat16, kind="ExternalInput")
    v1t = nc.dram_tensor("v1t", [nw, NFT, 128, FC, 8, 128],
                         mybir.dt.bfloat16, kind="ExternalInput")
    w2 = nc.dram_tensor("w2", [nw, NFT, 128, 4, H], mybir.dt.bfloat16,
                        kind="ExternalInput")
    coef = nc.dram_tensor("coef", [128, nblk], mybir.dt.float32,
                          kind="ExternalInput")
    yout = nc.dram_tensor("yout", [128, nblk, H], mybir.dt.bfloat16,
                          kind="ExternalOutput")

    with tile.TileContext(nc) as tc:
        with (
            tc.tile_pool(name="xpool", bufs=1) as xpool,
            tc.tile_pool(name="wpool", bufs=2) as wpool,
            tc.tile_pool(name="hpool", bufs=2) as hpool,
            tc.tile_pool(name="spool", bufs=2) as spool,
            tc.tile_pool(name="opool", bufs=1) as opool,
            tc.tile_pool(name="tpool", bufs=2) as tpool,
            tc.tile_pool(name="cpool", bufs=1) as cpool,
            tc.tile_pool(name="ps1", bufs=2, space="PSUM") as ps1,
            tc.tile_pool(name="ps2", bufs=2, space="PSUM") as ps2,
            tc.tile_pool(name="pso", bufs=4, space="PSUM") as psop,
        ):
            # PE warm-up burst: dummy matmuls during the initial DMA fill so
            # HAM un-throttles before real work.
            wu = cpool.tile([128, 128], mybir.dt.bfloat16)
            nc.gpsimd.memset(wu[:], 0.0)
            wups = ps1.tile([128, 512], mybir.dt.float32, tag="p1")
            for _ in range(90):
                nc.tensor.matmul(wups[:, :128], wu[:], wu[:],
                                 start=True, stop=True)

            coefs = cpool.tile([128, nblk], mybir.dt.float32)
            xts = xpool.tile([128, 8, cap], mybir.dt.bfloat16, tag="xts")
            # bf16 running accumulator over f-tiles (fp32 finish in tpool)
            oacc = opool.tile([128, nblk, H], mybir.dt.bfloat16, tag="oacc")

            for ft in range(NFT):
                w1s, v1s, w2s = [], [], []
                for w in range(nw):
                    wb = 2 if w == 0 else 1
                    w1s.append(wpool.tile([128, FC, 8, 128],
                                          mybir.dt.bfloat16,
                                          tag=f"w1s{w}", name=f"w1s{w}",
                                          bufs=wb))
                    v1s.append(wpool.tile([128, FC, 8, 128],
                                          mybir.dt.bfloat16,
                                          tag=f"v1s{w}", name=f"v1s{w}",
                                          bufs=wb))
                    w2s.append(wpool.tile([128, 4, H], mybir.dt.bfloat16,
                                          tag=f"w2s{w}", name=f"w2s{w}",
                                          bufs=wb))
                if ft == 0:
                    # startup: first token chunk + first weight fc-chunk
                    # land first; later chunks stream in while the first
                    # segment computes (stage-2 interleaving keeps the
                    # early bandwidth demand low)
                    nc.sync.dma_start(xts[:, :, 0:int(offs[1])],
                                      xt[:, :, 0:int(offs[1])])
                    nc.sync.dma_start(w1s[0][:, 0], w1t[0, ft, :, 0])
                    nc.sync.dma_start(v1s[0][:, 0], v1t[0, ft, :, 0])
                    for fc in range(1, FC):
                        nc.sync.dma_start(w1s[0][:, fc], w1t[0, ft, :, fc])
                        nc.sync.dma_start(v1s[0][:, fc], v1t[0, ft, :, fc])
                    nc.sync.dma_start(w2s[0][:], w2[0, ft])
                    for ((sz, _), o) in list(zip(layout, offs[:-1]))[1:]:
                        nc.sync.dma_start(xts[:, :, o:o + sz],
                                          xt[:, :, o:o + sz])
                    for w in range(1, nw):
                        nc.sync.dma_start(w1s[w][:], w1t[w, ft])
                        nc.sync.dma_start(v1s[w][:], v1t[w, ft])
                        nc.sync.dma_start(w2s[w][:], w2[w, ft])
                    nc.sync.dma_start(coefs[:], coef[:])
                else:
                    for w in range(nw):
                        nc.sync.dma_start(w1s[w][:], w1t[w, ft])
                        nc.sync.dma_start(v1s[w][:], v1t[w, ft])
                        nc.sync.dma_start(w2s[w][:], w2[w, ft])

                hmid = hpool.tile([128, FC, cap], mybir.dt.bfloat16,
                                  tag="hmid")
                # per segment: stage 1 over fc, then its stage-2 blocks --
                # interleaving stage 2 keeps the PE busy on already-loaded
                # data while later token chunks / weights stream in
                for (sz, w), o in zip(layout, offs[:-1]):
                    t0, tl = int(o), int(sz)
                    for fc in range(FC):
                        p1 = ps1.tile([128, 512], mybir.dt.float32)
                        p2 = ps2.tile([128, 512], mybir.dt.float32)
                        for hs in range(8):
                            nc.tensor.matmul(
                                p1[:, :tl], w1s[w][:, fc, hs],
                                xts[:, hs, t0:t0 + tl],
                                start=(hs == 0), stop=(hs == 7))
                        for hs in range(8):
                            nc.tensor.matmul(
                                p2[:, :tl], v1s[w][:, fc, hs],
                                xts[:, hs, t0:t0 + tl],
                                start=(hs == 0), stop=(hs == 7))
                        sil = spool.tile([128, 512], mybir.dt.float32)
                        nc.scalar.activation(
                            sil[:, :tl], p1[:, :tl],
                            mybir.ActivationFunctionType.Silu)
                        nc.vector.tensor_mul(
                            hmid[:, fc, t0:t0 + tl], sil[:, :tl],
                            p2[:, :tl])

                    nb = _ceil_div(tl, 128)
                    b0 = t0 // 128
                    for m in range(nb):
                        bi = b0 + m
                        mw = min(128, tl - m * 128)
                        msl = slice(t0 + m * 128, t0 + m * 128 + mw)
                        pos = [psop.tile([128, 512], mybir.dt.float32,
                                         tag="po", name=f"po{n}")
                               for n in range(NH)]
                        for fc in range(FC):  # lhsT reused across n chunks
                            for n in range(NH):
                                nc.tensor.matmul(
                                    pos[n][:mw], hmid[:, fc, msl],
                                    w2s[w][:, fc, n * 512:(n + 1) * 512],
                                    start=(fc == 0), stop=(fc == FC - 1))
                        if ft < NFT - 1:
                            for n in range(NH):
                                osl = oacc[:mw, bi, n * 512:(n + 1) * 512]
                                if ft == 0:
                                    nc.scalar.copy(osl, pos[n][:mw])
                                else:
                                    nc.vector.tensor_add(osl, osl,
                                                         pos[n][:mw])
                        else:
                            # finish: add + scale, single per-block bf16 DMA
                            # so the tail pipeline keeps up with the MMs
                            fin = tpool.tile([128, H], mybir.dt.bfloat16,
                                             bufs=4)
                            for n in range(NH):
                                nsl = slice(n * 512, (n + 1) * 512)
                                nc.vector.tensor_add(
                                    fin[:mw, nsl], oacc[:mw, bi, nsl],
                                    pos[n][:mw])
                            nc.vector.tensor_scalar_mul(
                                fin[:mw, :], fin[:mw, :],
                                coefs[:mw, bi:bi + 1])
                            nc.sync.dma_start(yout[:mw, bi, :], fin[:mw])

    nc.compile()
    return nc


def _get_nc(layout):
    if layout not in _compiled:
        _compiled[layout] = _build_nc(layout)
    return _compiled[layout]


# --------------------------------------------------------------------------
# host side: routing, packing, layout
# --------------------------------------------------------------------------

def _route(x, router_w):
    """Top-2 router, matching the reference (jax on CPU if available)."""
    try:
        import jax
        import jax.numpy as jnp
        cpu = jax.devices("cpu")[0]
        with jax.default_device(cpu):
            xl = jax.device_put(jnp.asarray(x), cpu)
            rw = jax.device_put(jnp.asarray(router_w), cpu)
            logits = xl @ rw.T
            scores = jax.nn.softmax(logits.astype(jnp.float32), axis=-1)
            ew, ei = jax.lax.top_k(scores, TOPK)
            ew = ew / ew.sum(axis=-1, keepdims=True)
            return np.asarray(ew, np.float32), np.asarray(ei, np.int64)
    except Exception:
        logits = x.astype(np.float32) @ router_w.astype(np.float32).T
        m = logits.max(axis=-1, keepdims=True)
        p = np.exp(logits - m)
        scores = (p / p.sum(axis=-1, keepdims=True)).astype(np.float32)
        i1 = scores.argmax(axis=-1)
        s2 = scores.copy()
        s2[np.arange(T), i1] = -np.inf
        i2 = s2.argmax(axis=-1)
        wa = scores[np.arange(T), i1]
        wb = scores[np.arange(T), i2]
        tot = wa + wb
        ew = np.stack([wa / tot, wb / tot], axis=-1).astype(np.float32)
        ei = np.stack([i1, i2], axis=-1).astype(np.int64)
        return ew, ei


def _pack(counts):
    """Big+spill packing.

    Layout (SPMD-uniform): 4x(512, wslot 0) main run + one (spill, wslot 1)
    chunk.  Each expert (desc count) gets its own core: first min(c_e, 2048)
    tokens fill the main run; overflow is cut into <=spill pieces placed in
    other cores' spill chunk.  Cores without a spill piece duplicate their
    main expert in wslot 1 with zero coef.

    Returns (layout, per_core_runs) with per_core_runs[c] a list of
    (wslot, tok_off, expert, n_tokens), or (None, None) if infeasible."""
    order = [int(e) for e in np.argsort(-counts) if counts[e] > 0]
    if len(order) > NCORES:
        return None, None
    spills = []
    runs = [[] for _ in range(NCORES)]
    for c, e in enumerate(order):
        rem = int(counts[e])
        runs[c].append((0, 0, e, min(rem, 2048)))
        if rem > 2048:
            spills.append([e, rem - 2048])
    if not spills:
        return tuple([(512, 0)] * 4), runs
    for spill_sz in (64, 128, 256, 512):
        pieces = []
        for e, rem in spills:
            n = _ceil_div(rem, spill_sz)
            pieces += [(e, min(spill_sz, rem - i * spill_sz))
                       for i in range(n)]
        if len(pieces) <= NCORES:
            for c, (e, n) in enumerate(pieces):
                runs[c].append((1, 2048, e, n))
            layout = tuple([(512, 0)] * 4 + [(spill_sz, 1)])
            return layout, runs
    return None, None


def _pack_fallback(counts):
    """General fallback: greedy bin-pack of experts onto 8 copies of a
    static slot template; each slot gets its own weight slot (old
    behaviour, weights re-streamed per slot)."""
    for tpl in ((512, 512, 512, 512, 512),
                (512,) * 6, (512,) * 8, (1024,) * 4, (2048,) * 3):
        slots = []
        for c in range(NCORES):
            for i, sz in enumerate(tpl):
                slots.append([sz, c, i, None, 0])
        free = sorted(range(len(slots)), key=lambda i: -slots[i][0])
        ok = True
        for e in np.argsort(-counts):
            rem = int(counts[e])
            while rem > 0:
                fit = [i for i in free if slots[i][0] >= rem]
                if fit:
                    pick = min(fit, key=lambda i: slots[i][0])
                elif free:
                    pick = free[0]
                else:
                    ok = False
                    break
                free.remove(pick)
                take = min(rem, slots[pick][0])
                slots[pick][3] = int(e)
                slots[pick][4] = take
                rem -= take
            if not ok:
                break
        if not ok:
            continue
        offs = np.concatenate([[0], np.cumsum(tpl)]).astype(int)
        runs = [[] for _ in range(NCORES)]
        for sz, c, i, e, used in slots:
            if e is not None:
                runs[c].append((i, int(offs[i]), e, used))
        layout = tuple((sz, i) for i, sz in enumerate(tpl))
        return layout, runs
    raise AssertionError("no feasible packing")


def _to_bf16(a):
    """Fast float32 -> bfloat16 with round-to-nearest-even."""
    u = np.ascontiguousarray(a, np.float32).view(np.uint32)
    r = ((u + np.uint32(0x7FFF) + ((u >> np.uint32(16)) & np.uint32(1)))
         >> np.uint32(16)).astype(np.uint16)
    return r.view(BF16)


def _prep_weights(w1, v1, w2):
    """Per-expert device layouts (bf16).

    w1t/v1t: [E][NFT,128,FC,8,128]  elem [ft,p,fc,hs,fl] =
                 W[ft*FT+fc*128+fl, hs*128+p]   (fc-major, contiguous chunks)
    w2     : [E][NFT,128,4,H]   elem [ft,p,fc,h] = w2[ft*FT+fc*128+p, h]
    """
    w1t, v1t, w2d = [], [], []
    for e in range(E):
        for src, dst in ((w1, w1t), (v1, v1t)):
            a = _to_bf16(src[e])                      # [F, H]
            a = np.ascontiguousarray(a.T)             # [H, F]
            a = a.reshape(8, 128, NFT, FC, 128).transpose(2, 1, 3, 0, 4)
            dst.append(np.ascontiguousarray(a))
        b = _to_bf16(w2[e])                           # [F, H]
        b = b.reshape(NFT, 4, 128, H).transpose(0, 2, 1, 3)
        w2d.append(np.ascontiguousarray(b))
    return w1t, v1t, w2d


def _forward(hidden_states, router_w, w1, v1, w2, trace=False):
    from concourse.bass_utils import run_bass_kernel_spmd

    x = np.ascontiguousarray(np.asarray(hidden_states, np.float32)).reshape(T, H)
    router_w = np.asarray(router_w, np.float32)
    w1 = np.asarray(w1, np.float32)
    v1 = np.asarray(v1, np.float32)
    w2 = np.asarray(w2, np.float32)

    ew, ei = _route(x, router_w)
    counts = np.bincount(ei.ravel(), minlength=E)
    layout, per_core_runs = _pack(counts)
    if layout is None:
        layout, per_core_runs = _pack_fallback(counts)
    sizes = [c[0] for c in layout]
    cap = sum(sizes)
    nw = max(w for _, w in layout) + 1
    offs = np.concatenate([[0], np.cumsum(sizes)]).astype(int)
    # block grid (must match device)
    blocks = []
    for (sz, w), o in zip(layout, offs[:-1]):
        t0 = 0
        while t0 < sz:
            mw = min(128, sz - t0)
            blocks.append((int(o + t0), mw, w))
            t0 += mw
    nblk = len(blocks)

    # per-expert assignment lists (token ids + weights), then cursors
    flat_e = ei.ravel()
    flat_w = ew.ravel().astype(np.float32)
    order = np.argsort(flat_e, kind="stable")
    toks_s = (order // TOPK).astype(np.int64)
    ws_s = flat_w[order]
    starts = np.concatenate([[0], np.cumsum(counts)]).astype(int)
    cursor = starts[:-1].copy()

    w1t_pre, v1t_pre, w2_pre = _prep_weights(w1, v1, w2)
    xbf = _to_bf16(x)  # [T, H] bf16

    in_maps = []
    core_lists = []  # per core: list of (tok_off, ids) for scatter
    for c in range(NCORES):
        xt_np = np.zeros((128, 8, cap), BF16)
        w1t_np = np.zeros((nw, NFT, 128, FC, 8, 128), BF16)
        v1t_np = np.zeros((nw, NFT, 128, FC, 8, 128), BF16)
        w2_np = np.zeros((nw, NFT, 128, 4, H), BF16)
        coef_np = np.zeros((128, nblk), np.float32)
        lists = []
        filled = set()
        for wslot, tok_off, e, used in per_core_runs[c]:
            ids = toks_s[cursor[e]:cursor[e] + used]
            ws = ws_s[cursor[e]:cursor[e] + used]
            cursor[e] += used
            L = used
            xg = np.ascontiguousarray(xbf[ids].T)     # [H, L]
            xt_np[:, :, tok_off:tok_off + L] = \
                xg.reshape(8, 128, L).transpose(1, 0, 2)
            nm = _ceil_div(L, 128)
            wpad = np.zeros(nm * 128, np.float32)
            wpad[:L] = ws
            b0 = tok_off // 128  # runs start 128-aligned
            coef_np[:, b0:b0 + nm] = wpad.reshape(nm, 128).T
            w1t_np[wslot] = w1t_pre[e]
            v1t_np[wslot] = v1t_pre[e]
            w2_np[wslot] = w2_pre[e]
            filled.add(wslot)
            lists.append((tok_off, ids))
        # unused weight slots: duplicate expert 0 weights (coef stays 0,
        # so the computed garbage is multiplied by zero -- but weights must
        # be finite)
        for wslot in range(nw):
            if wslot not in filled:
                w1t_np[wslot] = w1t_pre[0]
                v1t_np[wslot] = v1t_pre[0]
                w2_np[wslot] = w2_pre[0]
        core_lists.append(lists)
        in_maps.append({"xt": xt_np, "w1t": w1t_np, "v1t": v1t_np,
                        "w2": w2_np, "coef": coef_np})
    assert (cursor == starts[1:]).all()

    nc = _get_nc(layout)
    if trace:
        _install_profile_shim()
    res = run_bass_kernel_spmd(nc, in_maps, list(range(NCORES)), trace=trace)

    out = np.zeros((T, H), np.float32)
    for c in range(NCORES):
        y = res.results[c]["yout"]  # [128, nblk, H] bf16
        yflat = np.asarray(y, np.float32).transpose(1, 0, 2) \
            .reshape(nblk * 128, H)
        for tok_off, ids in core_lists[c]:
            L = len(ids)
            out[ids] += yflat[tok_off:tok_off + L]
    return out.reshape(B, S, H), res


def kernel(hidden_states, router_w, w1, v1, w2):
    out, _ = _forward(hidden_states, router_w, w1, v1, w2, trace=False)
    return out


def _install_profile_shim():
    """The agent image's antenv lacks axon_hooks; register the NTFF
    profile hook from trn_agent_boot so trace=True works."""
    import sys
    import types
    if "antenv.axon_hooks" in sys.modules:
        return
    holder = {}
    mod = types.ModuleType("antenv.axon_hooks")
    mod.set_axon_ntff_profile_hook = lambda h: holder.__setitem__("h", h)
    mod.get_axon_ntff_profile_hook = lambda: holder.get("h")
    sys.modules["antenv.axon_hooks"] = mod
    try:
        from trn_agent_boot.trn_boot import _ntff_profile_via_ctypes
        hook = _ntff_profile_via_ctypes("/opt/axon/libaxon_pjrt.so")
        mod.set_axon_ntff_profile_hook(hook)
    except Exception as exc:  # pragma: no cover
        print(f"profile shim failed: {exc}")


# revision 25
# speedup vs baseline: 1.1981x; 1.0007x over previous
"""Mixtral MoE (top-2 of 8 experts, GLU) on 8 Trainium2 cores.  v4.

Structure (per core, SPMD-uniform):
  - tokens laid out as a flat [128, 8, cap] block; "chunks" (<=512 tokens)
    for stage 1 and 128-token "blocks" for stage 2, each statically mapped
    to a weight slot (wslot).  The standard layout is a 2048-token main run
    (wslot 0) + one small spill chunk (wslot 1), so each expert's weights
    stream once per f-tile instead of once per 512-slot (4x less SBUF-write
    DMA traffic -> fewer PE stalls from port contention).
  - loop: ft outer; stage 1 (all chunks) -> hmid; stage 2 (all blocks)
    accumulates into a bf16 oacc; final ft adds in fp32, scales by coef and
    streams out.
"""

import numpy as np
import ml_dtypes

B, S, H, F, E, TOPK = 4, 2048, 1024, 3584, 8, 2
T = B * S
NCORES = 8
NFT = 7                # F tiles
FT = F // NFT          # 512
FC = FT // 128         # 4
NH = H // 512          # 2
BF16 = ml_dtypes.bfloat16

_compiled = {}


def _ceil_div(a, b):
    return -(-a // b)


# --------------------------------------------------------------------------
# device kernel
# --------------------------------------------------------------------------

def _build_nc(layout):
    """layout: tuple of (chunk_size, wslot) pairs; chunk starts must keep
    128-token blocks within a single wslot (sizes multiple of 128 except
    possibly the last chunk of a wslot run)."""
    import concourse.tile as tile
    import concourse.mybir as mybir
    from concourse import bacc

    sizes = [c[0] for c in layout]
    wslots = [c[1] for c in layout]
    nw = max(wslots) + 1
    cap = sum(sizes)
    offs = np.concatenate([[0], np.cumsum(sizes)]).astype(int)
    # stage-2 blocks: global 128-grid; each block must lie inside one chunk
    blocks = []   # (tok0, mw, wslot)
    for (sz, w), o in zip(layout, offs[:-1]):
        t0 = 0
        while t0 < sz:
            mw = min(128, sz - t0)
            blocks.append((int(o + t0), mw, w))
            t0 += mw
    nblk = len(blocks)

    nc = bacc.Bacc("TRN2", target_bir_lowering=False, debug=False,
                   num_devices=NCORES)
    xt = nc.dram_tensor("xt", [128, 8, cap], mybir.dt.bfloat16,
                        kind="ExternalInput")
    # fc-major so per-fc startup chunks are contiguous in DRAM
    w1t = nc.dram_tensor("w1t", [nw, NFT, 128, FC, 8, 128],
                         mybir.dt.bfloat16, kind="ExternalInput")
    v1t = nc.dram_tensor("v1t", [nw, NFT, 128, FC, 8, 128],
                         mybir.dt.bfloat16, kind="ExternalInput")
    w2 = nc.dram_tensor("w2", [nw, NFT, 128, 4, H], mybir.dt.bfloat16,
                        kind="ExternalInput")
    coef = nc.dram_tensor("coef", [128, nblk], mybir.dt.float32,
                          kind="ExternalInput")
    yout = nc.dram_tensor("yout", [128, nblk, H], mybir.dt.bfloat16,
                          kind="ExternalOutput")

    with tile.TileContext(nc) as tc:
        with (
            tc.tile_pool(name="xpool", bufs=1) as xpool,
            tc.tile_pool(name="wpool", bufs=2) as wpool,
            tc.tile_pool(name="hpool", bufs=2) as hpool,
            tc.tile_pool(name="spool", bufs=2) as spool,
            tc.tile_pool(name="opool", bufs=1) as opool,
            tc.tile_pool(name="tpool", bufs=2) as tpool,
            tc.tile_pool(name="cpool", bufs=1) as cpool,
            tc.tile_pool(name="ps1", bufs=2, space="PSUM") as ps1,
            tc.tile_pool(name="ps2", bufs=2, space="PSUM") as ps2,
            tc.tile_pool(name="pso", bufs=4, space="PSUM") as psop,
        ):
            # PE warm-up burst: dummy matmuls during the initial DMA fill so
            # HAM un-throttles before real work.
            wu = cpool.tile([128, 128], mybir.dt.bfloat16)
            nc.gpsimd.memset(wu[:], 0.0)
            wups = ps1.tile([128, 512], mybir.dt.float32, tag="p1")
            for _ in range(90):
                nc.tensor.matmul(wups[:, :128], wu[:], wu[:],
                                 start=True, stop=True)

            coefs = cpool.tile([128, nblk], mybir.dt.float32)
            xts = xpool.tile([128, 8, cap], mybir.dt.bfloat16, tag="xts")
            # bf16 running accumulator over f-tiles (fp32 finish in tpool)
            oacc = opool.tile([128, nblk, H], mybir.dt.bfloat16, tag="oacc")

            for ft in range(NFT):
                w1s, v1s, w2s = [], [], []
                for w in range(nw):
                    wb = 2 if w == 0 else 1
                    w1s.append(wpool.tile([128, FC, 8, 128],
                                          mybir.dt.bfloat16,
                                          tag=f"w1s{w}", name=f"w1s{w}",
                                          bufs=wb))
                    v1s.append(wpool.tile([128, FC, 8, 128],
                                          mybir.dt.bfloat16,
                                          tag=f"v1s{w}", name=f"v1s{w}",
                                          bufs=wb))
                    w2s.append(wpool.tile([128, 4, H], mybir.dt.bfloat16,
                                          tag=f"w2s{w}", name=f"w2s{w}",
                                          bufs=wb))
                if ft == 0:
                    # startup: first token chunk + first weight fc-chunk
                    # land first; later chunks stream in while the first
                    # segment computes (stage-2 interleaving keeps the
                    # early bandwidth demand low)
                    nc.sync.dma_start(xts[:, :, 0:int(offs[1])],
                                      xt[:, :, 0:int(offs[1])])
                    nc.sync.dma_start(w1s[0][:, 0], w1t[0, ft, :, 0])
                    nc.sync.dma_start(v1s[0][:, 0], v1t[0, ft, :, 0])
                    for fc in range(1, FC):
                        nc.sync.dma_start(w1s[0][:, fc], w1t[0, ft, :, fc])
                        nc.sync.dma_start(v1s[0][:, fc], v1t[0, ft, :, fc])
                    nc.sync.dma_start(w2s[0][:], w2[0, ft])
                    for ((sz, _), o) in list(zip(layout, offs[:-1]))[1:]:
                        nc.sync.dma_start(xts[:, :, o:o + sz],
                                          xt[:, :, o:o + sz])
                    for w in range(1, nw):
                        nc.sync.dma_start(w1s[w][:], w1t[w, ft])
                        nc.sync.dma_start(v1s[w][:], v1t[w, ft])
                        nc.sync.dma_start(w2s[w][:], w2[w, ft])
                    nc.sync.dma_start(coefs[:], coef[:])
                else:
                    for w in range(nw):
                        nc.sync.dma_start(w1s[w][:], w1t[w, ft])
                        nc.sync.dma_start(v1s[w][:], v1t[w, ft])
                        nc.sync.dma_start(w2s[w][:], w2[w, ft])

                hmid = hpool.tile([128, FC, cap], mybir.dt.bfloat16,
                                  tag="hmid")
                # per segment: stage 1 over fc, then its stage-2 blocks --
                # interleaving stage 2 keeps the PE busy on already-loaded
                # data while later token chunks / weights stream in
                for (sz, w), o in zip(layout, offs[:-1]):
                    t0, tl = int(o), int(sz)
                    for fc in range(FC):
                        p1 = ps1.tile([128, 512], mybir.dt.float32)
                        p2 = ps2.tile([128, 512], mybir.dt.float32)
                        for hs in range(8):
                            nc.tensor.matmul(
                                p1[:, :tl], w1s[w][:, fc, hs],
                                xts[:, hs, t0:t0 + tl],
                                start=(hs == 0), stop=(hs == 7))
                        for hs in range(8):
                            nc.tensor.matmul(
                                p2[:, :tl], v1s[w][:, fc, hs],
                                xts[:, hs, t0:t0 + tl],
                                start=(hs == 0), stop=(hs == 7))
                        sil = spool.tile([128, 512], mybir.dt.float32)
                        nc.scalar.activation(
                            sil[:, :tl], p1[:, :tl],
                            mybir.ActivationFunctionType.Silu)
                        nc.vector.tensor_mul(
                            hmid[:, fc, t0:t0 + tl], sil[:, :tl],
                            p2[:, :tl])

                    nb = _ceil_div(tl, 128)
                    b0 = t0 // 128
                    for m in range(nb):
                        bi = b0 + m
                        mw = min(128, tl - m * 128)
                        msl = slice(t0 + m * 128, t0 + m * 128 + mw)
                        pos = [psop.tile([128, 512], mybir.dt.float32,
                                         tag="po", name=f"po{n}")
                               for n in range(NH)]
                        for fc in range(FC):  # lhsT reused across n chunks
                            for n in range(NH):
                                nc.tensor.matmul(
                                    pos[n][:mw], hmid[:, fc, msl],
                                    w2s[w][:, fc, n * 512:(n + 1) * 512],
                                    start=(fc == 0), stop=(fc == FC - 1))
                        if ft < NFT - 1:
                            for n in range(NH):
                                osl = oacc[:mw, bi, n * 512:(n + 1) * 512]
                                if ft == 0:
                                    nc.scalar.copy(osl, pos[n][:mw])
                                else:
                                    nc.vector.tensor_add(osl, osl,
                                                         pos[n][:mw])
                        else:
                            # finish: add + scale, single per-block bf16 DMA
                            # so the tail pipeline keeps up with the MMs
                            fin = tpool.tile([128, H], mybir.dt.bfloat16,
                                             bufs=4)
                            for n in range(NH):
                                nsl = slice(n * 512, (n + 1) * 512)
                                nc.vector.tensor_add(
                                    fin[:mw, nsl], oacc[:mw, bi, nsl],
                                    pos[n][:mw])
                            nc.vector.tensor_scalar_mul(
                                fin[:mw, :], fin[:mw, :],
                                coefs[:mw, bi:bi + 1])
                            nc.sync.dma_start(yout[:mw, bi, :], fin[:mw])

    nc.compile()
    return nc


def _get_nc(layout):
    if layout not in _compiled:
        _compiled[layout] = _build_nc(layout)
    return _compiled[layout]


# --------------------------------------------------------------------------
# host side: routing, packing, layout
# --------------------------------------------------------------------------

def _route(x, router_w):
    """Top-2 router, matching the reference (jax on CPU if available)."""
    try:
        import jax
        import jax.numpy as jnp
        cpu = jax.devices("cpu")[0]
        with jax.default_device(cpu):
            xl = jax.device_put(jnp.asarray(x), cpu)
            rw = jax.device_put(jnp.asarray(router_w), cpu)
            logits = xl @ rw.T
            scores = jax.nn.softmax(logits.astype(jnp.float32), axis=-1)
            ew, ei = jax.lax.top_k(scores, TOPK)
            ew = ew / ew.sum(axis=-1, keepdims=True)
            return np.asarray(ew, np.float32), np.asarray(ei, np.int64)
    except Exception:
        logits = x.astype(np.float32) @ router_w.astype(np.float32).T
        m = logits.max(axis=-1, keepdims=True)
        p = np.exp(logits - m)
        scores = (p / p.sum(axis=-1, keepdims=True)).astype(np.float32)
        i1 = scores.argmax(axis=-1)
        s2 = scores.copy()
        s2[np.arange(T), i1] = -np.inf
        i2 = s2.argmax(axis=-1)
        wa = scores[np.arange(T), i1]
        wb = scores[np.arange(T), i2]
        tot = wa + wb
        ew = np.stack([wa / tot, wb / tot], axis=-1).astype(np.float32)
        ei = np.stack([i1, i2], axis=-1).astype(np.int64)
        return ew, ei


def _pack(counts):
    """Big+spill packing.

    Layout (SPMD-uniform): 4x(512, wslot 0) main run + one (spill, wslot 1)
    chunk.  Each expert (desc count) gets its own core: first min(c_e, 2048)
    tokens fill the main run; overflow is cut into <=spill pieces placed in
    other cores' spill chunk.  Cores without a spill piece duplicate their
    main expert in wslot 1 with zero coef.

    Returns (layout, per_core_runs) with per_core_runs[c] a list of
    (wslot, tok_off, expert, n_tokens), or (None, None) if infeasible."""
    order = [int(e) for e in np.argsort(-counts) if counts[e] > 0]
    if len(order) > NCORES:
        return None, None
    spills = []
    runs = [[] for _ in range(NCORES)]
    for c, e in enumerate(order):
        rem = int(counts[e])
        runs[c].append((0, 0, e, min(rem, 2048)))
        if rem > 2048:
            spills.append([e, rem - 2048])
    if not spills:
        return tuple([(512, 0)] * 4), runs
    for spill_sz in (64, 128, 256, 512):
        pieces = []
        for e, rem in spills:
            n = _ceil_div(rem, spill_sz)
            pieces += [(e, min(spill_sz, rem - i * spill_sz))
                       for i in range(n)]
        if len(pieces) <= NCORES:
            for c, (e, n) in enumerate(pieces):
                runs[c].append((1, 2048, e, n))
            layout = tuple([(512, 0)] * 4 + [(spill_sz, 1)])
            return layout, runs
    return None, None


def _pack_fallback(counts):
    """General fallback: greedy bin-pack of experts onto 8 copies of a
    static slot template; each slot gets its own weight slot (old
    behaviour, weights re-streamed per slot)."""
    for tpl in ((512, 512, 512, 512, 512),
                (512,) * 6, (512,) * 8, (1024,) * 4, (2048,) * 3):
        slots = []
        for c in range(NCORES):
            for i, sz in enumerate(tpl):
                slots.append([sz, c, i, None, 0])
        free = sorted(range(len(slots)), key=lambda i: -slots[i][0])
        ok = True
        for e in np.argsort(-counts):
            rem = int(counts[e])
            while rem > 0:
                fit = [i for i in free if slots[i][0] >= rem]
                if fit:
                    pick = min(fit, key=lambda i: slots[i][0])
                elif free:
                    pick = free[0]
                else:
                    ok = False
                    break
                free.remove(pick)
                take = min(rem, slots[pick][0])
                slots[pick][3] = int(e)
                slots[pick][4] = take
                rem -= take
            if not ok:
                break
        if not ok:
            continue
        offs = np.concatenate([[0], np.cumsum(tpl)]).astype(int)
        runs = [[] for _ in range(NCORES)]
        for sz, c, i, e, used in slots:
            if e is not None:
                runs[c].append((i, int(offs[i]), e, used))
        layout = tuple((sz, i) for i, sz in enumerate(tpl))
        return layout, runs
    raise AssertionError("no feasible packing")


def _to_bf16(a):
    """Fast float32 -> bfloat16 with round-to-nearest-even."""
    u = np.ascontiguousarray(a, np.float32).view(np.uint32)
    r = ((u + np.uint32(0x7FFF) + ((u >> np.uint32(16)) & np.uint32(1)))
         >> np.uint32(16)).astype(np.uint16)
    return r.view(BF16)


def _prep_weights(w1, v1, w2):
    """Per-expert device layouts (bf16).

    w1t/v1t: [E][NFT,128,FC,8,128]  elem [ft,p,fc,hs,fl] =
                 W[ft*FT+fc*128+fl, hs*128+p]   (fc-major, contiguous chunks)
    w2     : [E][NFT,128,4,H]   elem [ft,p,fc,h] = w2[ft*FT+fc*128+p, h]
    """
    w1t, v1t, w2d = [], [], []
    for e in range(E):
        for src, dst in ((w1, w1t), (v1, v1t)):
            a = _to_bf16(src[e])                      # [F, H]
            a = np.ascontiguousarray(a.T)             # [H, F]
            a = a.reshape(8, 128, NFT, FC, 128).transpose(2, 1, 3, 0, 4)
            dst.append(np.ascontiguousarray(a))
        b = _to_bf16(w2[e])                           # [F, H]
        b = b.reshape(NFT, 4, 128, H).transpose(0, 2, 1, 3)
        w2d.append(np.ascontiguousarray(b))
    return w1t, v1t, w2d


def _forward(hidden_states, router_w, w1, v1, w2, trace=False):
    from concourse.bass_utils import run_bass_kernel_spmd

    x = np.ascontiguousarray(np.asarray(hidden_states, np.float32)).reshape(T, H)
    router_w = np.asarray(router_w, np.float32)
    w1 = np.asarray(w1, np.float32)
    v1 = np.asarray(v1, np.float32)
    w2 = np.asarray(w2, np.float32)

    ew, ei = _route(x, router_w)
    counts = np.bincount(ei.ravel(), minlength=E)
    layout, per_core_runs = _pack(counts)
    if layout is None:
        layout, per_core_runs = _pack_fallback(counts)
    sizes = [c[0] for c in layout]
    cap = sum(sizes)
    nw = max(w for _, w in layout) + 1
    offs = np.concatenate([[0], np.cumsum(sizes)]).astype(int)
    # block grid (must match device)
    blocks = []
    for (sz, w), o in zip(layout, offs[:-1]):
        t0 = 0
        while t0 < sz:
            mw = min(128, sz - t0)
            blocks.append((int(o + t0), mw, w))
            t0 += mw
    nblk = len(blocks)

    # per-expert assignment lists (token ids + weights), then cursors
    flat_e = ei.ravel()
    flat_w = ew.ravel().astype(np.float32)
    order = np.argsort(flat_e, kind="stable")
    toks_s = (order // TOPK).astype(np.int64)
    ws_s = flat_w[order]
    starts = np.concatenate([[0], np.cumsum(counts)]).astype(int)
    cursor = starts[:-1].copy()

    w1t_pre, v1t_pre, w2_pre = _prep_weights(w1, v1, w2)
    xbf = _to_bf16(x)  # [T, H] bf16

    in_maps = []
    core_lists = []  # per core: list of (tok_off, ids) for scatter
    for c in range(NCORES):
        xt_np = np.zeros((128, 8, cap), BF16)
        w1t_np = np.zeros((nw, NFT, 128, FC, 8, 128), BF16)
        v1t_np = np.zeros((nw, NFT, 128, FC, 8, 128), BF16)
        w2_np = np.zeros((nw, NFT, 128, 4, H), BF16)
        coef_np = np.zeros((128, nblk), np.float32)
        lists = []
        filled = set()
        for wslot, tok_off, e, used in per_core_runs[c]:
            ids = toks_s[cursor[e]:cursor[e] + used]
            ws = ws_s[cursor[e]:cursor[e] + used]
            cursor[e] += used
            L = used
            xg = np.ascontiguousarray(xbf[ids].T)     # [H, L]
            xt_np[:, :, tok_off:tok_off + L] = \
                xg.reshape(8, 128, L).transpose(1, 0, 2)
            nm = _ceil_div(L, 128)
            wpad = np.zeros(nm * 128, np.float32)
            wpad[:L] = ws
            b0 = tok_off // 128  # runs start 128-aligned
            coef_np[:, b0:b0 + nm] = wpad.reshape(nm, 128).T
            w1t_np[wslot] = w1t_pre[e]
            v1t_np[wslot] = v1t_pre[e]
            w2_np[wslot] = w2_pre[e]
            filled.add(wslot)
            lists.append((tok_off, ids))
        # unused weight slots: duplicate expert 0 weights (coef stays 0,
        # so the computed garbage is multiplied by zero -- but weights must
        # be finite)
        for wslot in range(nw):
            if wslot not in filled:
                w1t_np[wslot] = w1t_pre[0]
                v1t_np[wslot] = v1t_pre[0]
                w2_np[wslot] = w2_pre[0]
        core_lists.append(lists)
        in_maps.append({"xt": xt_np, "w1t": w1t_np, "v1t": v1t_np,
                        "w2": w2_np, "coef": coef_np})
    assert (cursor == starts[1:]).all()

    nc = _get_nc(layout)
    if trace:
        _install_profile_shim()
    res = run_bass_kernel_spmd(nc, in_maps, list(range(NCORES)), trace=trace)

    out = np.zeros((T, H), np.float32)
    for c in range(NCORES):
        y = res.results[c]["yout"]  # [128, nblk, H] bf16
        yflat = np.asarray(y, np.float32).transpose(1, 0, 2) \
            .reshape(nblk * 128, H)
        for tok_off, ids in core_lists[c]:
            L = len(ids)
            out[ids] += yflat[tok_off:tok_off + L]
    return out.reshape(B, S, H), res


def kernel(hidden_states, router_w, w1, v1, w2):
    out, _ = _forward(hidden_states, router_w, w1, v1, w2, trace=False)
    return out


def _install_profile_shim():
    """The agent image's antenv lacks axon_hooks; register the NTFF
    profile hook from trn_agent_boot so trace=True works."""
    import sys
    import types
    if "antenv.axon_hooks" in sys.modules:
        return
    holder = {}
    mod = types.ModuleType("antenv.axon_hooks")
    mod.set_axon_ntff_profile_hook = lambda h: holder.__setitem__("h", h)
    mod.get_axon_ntff_profile_hook = lambda: holder.get("h")
    sys.modules["antenv.axon_hooks"] = mod
    try:
        from trn_agent_boot.trn_boot import _ntff_profile_via_ctypes
        hook = _ntff_profile_via_ctypes("/opt/axon/libaxon_pjrt.so")
        mod.set_axon_ntff_profile_hook(hook)
    except Exception as exc:  # pragma: no cover
        print(f"profile shim failed: {exc}")
